# revision 1
# baseline (speedup 1.0000x reference)
"""Trainium2 Bass kernel for single-head cross-attention with additive mask.

Computation (matches the reference):
    q = tgt @ wq + bq
    k = src @ wk (+ bk dropped: softmax cancels a per-row constant exactly)
    v = src @ wv (bv folded into the epilogue: out = attn@v + bv)
    s = (q k^T + mask) / sqrt(DQ)
    out = softmax(s) @ v + bv

Two SPMD launches on 8 cores:
  L1: each core projects kT (fp32 psum) and v (fp16 matmuls) for 1/8 of the
      global (B*S) src rows from a host-pre-transposed src slice.
  host: concatenates the 8 K/V shards, appends the softmax-denominator ones
      column to V, casts K/V to fp16 (pure layout glue, no math).
  L2: tgt sharded 8 ways; core c handles tgt rows [c*512,(c+1)*512) of every
      batch so its 8MB mask slice is read from HBM exactly once.

Scores are built transposed (src rows on PSUM partitions) so the PV matmul
consumes softmax weights directly, batch-pair outer so the QK psum tile can
triple-buffer. Projections accumulate in fp32; Q/K/V/mask/P are fp16 (11
mantissa bits keeps rel-err ~1e-3 at full matmul speed). The mask is added
by the otherwise-idle DVE (fp16-cast on the fly by the load DMA), exp() runs
on ACT and emits fp16 attention weights, PV accumulates fp32 in PSUM, and
the epilogue (1/rowsum scaling + bv bias) is PE-free via gpsimd
partition_broadcast. The output leaves transposed [B, DQ, TS]; the host
flips it.
"""
import numpy as np

B, S, D, DQ = 4, 4096, 1024, 64
NCORES = 8
TS = S // NCORES            # 512 tgt rows per core
SR = (B * S) // NCORES      # 2048 global src rows per core (L1)
SB = S // 128               # 32 src blocks per batch
GK = B * SB                 # 128 global src blocks
CORES = list(range(NCORES))
F32 = np.float32
FP16 = np.float16

_CACHE = {}


def _build_l1():
    import concourse.mybir as mybir
    import concourse.tile as tile
    from concourse import bacc

    f32 = mybir.dt.float32
    fp16 = mybir.dt.float16

    nc = bacc.Bacc("TRN2", target_bir_lowering=False, debug=False,
                   num_devices=NCORES)
    srcT = nc.dram_tensor("srcT", [D, SR], f32, kind="ExternalInput")
    wk = nc.dram_tensor("wk", [D, DQ], f32, kind="ExternalInput")
    wv = nc.dram_tensor("wv", [D, DQ], f32, kind="ExternalInput")
    kt = nc.dram_tensor("kt", [DQ, 2, 1024], f32, kind="ExternalOutput")
    vout = nc.dram_tensor("vout", [SR, DQ], f32, kind="ExternalOutput")

    with tile.TileContext(nc) as tc:
        with (
            tc.tile_pool(name="const", bufs=1) as constp,
            tc.tile_pool(name="big", bufs=1) as bigp,
            tc.tile_pool(name="stream", bufs=2) as streamp,
            tc.tile_pool(name="pp", bufs=1, space="PSUM") as pp,
        ):
            wk_sb = constp.tile([128, 8 * DQ], f32)
            nc.sync.dma_start(
                out=wk_sb.rearrange("p (j m) -> p j m", m=DQ),
                in_=wk.rearrange("(j p) m -> p j m", p=128))
            wv_bf = constp.tile([128, 8 * DQ], fp16)
            nc.gpsimd.dma_start(
                out=wv_bf.rearrange("p (j m) -> p j m", m=DQ),
                in_=wv.rearrange("(j p) m -> p j m", p=128))

            kT_psA = pp.tile([128, 1024], f32, tag="qk0")
            kT_psB = pp.tile([128, 1024], f32, tag="qk1")
            v_ps = [pp.tile([128, 4 * DQ], f32, tag=f"pv{q}", name=f"v_ps{q}")
                    for q in range(4)]
            for j in range(8):
                st = streamp.tile([128, SR], f32, tag="xs", bufs=3)
                nc.sync.dma_start(out=st[:], in_=srcT[j * 128:(j + 1) * 128, :])
                stb = streamp.tile([128, SR], fp16, tag="xsb")
                nc.vector.tensor_copy(stb[:], st[:])
                for g in (0, 2, 1, 3):  # alternate col-groups for PE overlap
                    if g < 2:
                        ps, col, tp, po = kT_psA, g * 512, (0, 0), 0
                    else:
                        ps, col, tp, po = kT_psB, (g - 2) * 512, (0, 64), 64
                    nc.tensor.matmul(
                        ps[po:po + 64, col:col + 512],
                        lhsT=wk_sb[:, j * DQ:(j + 1) * DQ],
                        rhs=st[:, g * 512:(g + 1) * 512],
                        start=(j == 0), stop=(j == 7), tile_position=tp)
                for k in range(16):
                    nc.tensor.matmul(
                        v_ps[k // 4][:, (k % 4) * DQ:(k % 4 + 1) * DQ],
                        lhsT=stb[:, k * 128:(k + 1) * 128],
                        rhs=wv_bf[:, j * DQ:(j + 1) * DQ],
                        start=(j == 0 and k % 4 == 0),
                        stop=(j == 7 and k % 4 == 3))
            kT_sb = bigp.tile([128, 1024], f32)
            nc.scalar.copy(kT_sb[0:64, :], kT_psA[0:64, :])
            nc.scalar.copy(kT_sb[64:128, :], kT_psB[64:128, :])
            v_sb = bigp.tile([128, 16 * DQ], f32)
            for q in range(4):
                nc.vector.tensor_copy(v_sb[:, q * 256:(q + 1) * 256], v_ps[q][:])
            nc.sync.dma_start(out=kt[:, 0, :], in_=kT_sb[0:64, :])
            nc.sync.dma_start(out=kt[:, 1, :], in_=kT_sb[64:128, :])
            nc.gpsimd.dma_start(
                out=vout.rearrange("(k p) d -> p k d", p=128),
                in_=v_sb.rearrange("p (k d) -> p k d", d=DQ))
    nc.compile()
    return nc


def _build_l2():
    import concourse.mybir as mybir
    import concourse.tile as tile
    from concourse import bacc
    from concourse.masks import make_identity

    f32 = mybir.dt.float32
    fp16 = mybir.dt.float16
    AF = mybir.ActivationFunctionType

    nc = bacc.Bacc("TRN2", target_bir_lowering=False, debug=False,
                   num_devices=NCORES)
    # kT2 layout: partitions 0-63 = d, s of batches 0-1; 64-127 = batches 2-3
    kt2d = nc.dram_tensor("kt2", [128, 2 * S], fp16, kind="ExternalInput")
    # v65 in SBUF layout: row p, cols (k, c): element = v[k*128 + p, c] | ones
    v65d = nc.dram_tensor("v65", [128, GK * (DQ + 1)], fp16, kind="ExternalInput")
    tgtT = nc.dram_tensor("tgtT", [B, D, TS], f32, kind="ExternalInput")
    # host-transposed mask slice: masknT[s, t] = mask[c*TS + t, s]
    masknT = nc.dram_tensor("masknT", [S, TS], f32, kind="ExternalInput")
    wq = nc.dram_tensor("wq", [D, DQ], f32, kind="ExternalInput")
    bq = nc.dram_tensor("bq", [DQ], f32, kind="ExternalInput")
    bv = nc.dram_tensor("bv", [DQ], f32, kind="ExternalInput")
    # transposed output: host flips [B, DQ, TS] -> [B, TS, DQ]
    out = nc.dram_tensor("out", [B, DQ, TS], f32, kind="ExternalOutput")

    with tile.TileContext(nc) as tc:
        with (
            tc.tile_pool(name="const", bufs=1) as constp,
            tc.tile_pool(name="big", bufs=1) as bigp,
            tc.tile_pool(name="stream", bufs=2) as streamp,
            tc.tile_pool(name="pp", bufs=1, space="PSUM") as pp,
        ):
            wq_sb = constp.tile([128, 8 * DQ], f32)
            nc.sync.dma_start(
                out=wq_sb.rearrange("p (j m) -> p j m", m=DQ),
                in_=wq.rearrange("(j p) m -> p j m", p=128))
            bq_sb = constp.tile([128, 1], f32)
            nc.sync.dma_start(out=bq_sb[0:64, :], in_=bq.rearrange("(p o) -> p o", o=1))
            nc.sync.dma_start(out=bq_sb[64:128, :], in_=bq.rearrange("(p o) -> p o", o=1))
            bv_sb = constp.tile([64, 1], f32)
            nc.sync.dma_start(out=bv_sb[:], in_=bv.rearrange("(p o) -> p o", o=1))

            # resident loads, chunked so sg=0 unblocks early
            kT2 = bigp.tile([128, 2 * S], fp16)
            for q4 in (0, 2, 1, 3):  # first halves of both batch-halves first
                nc.sync.dma_start(out=kT2[:, q4 * 2048:(q4 + 1) * 2048],
                                  in_=kt2d[:, q4 * 2048:(q4 + 1) * 2048])
            v2 = bigp.tile([128, GK * (DQ + 1)], fp16)
            VQ = 32 * (DQ + 1)
            for q4 in range(4):
                nc.gpsimd.dma_start(out=v2[:, q4 * VQ:(q4 + 1) * VQ],
                                    in_=v65d[:, q4 * VQ:(q4 + 1) * VQ])
            # maskT, fp16-cast on the fly, duplicated per batch-half so one
            # [128, 1024] DVE add covers a whole score-pair tile:
            # layout [128 s-partitions, (sg, half, t)]
            maskTd = bigp.tile([128, SB * 2 * TS], fp16)
            mview = maskTd.rearrange("p (sb h t) -> p sb h t", h=2, t=TS)
            for g in range(4):
                nc.gpsimd.dma_start(
                    out=mview[:, g * 8:(g + 1) * 8, 0, :],
                    in_=masknT[g * 1024:(g + 1) * 1024, :]
                    .rearrange("(sb p) t -> p sb t", p=128))
                nc.vector.tensor_copy(mview[:, g * 8:(g + 1) * 8, 1, :],
                                      mview[:, g * 8:(g + 1) * 8, 0, :])

            # qT projection (fp32 matmuls, fp16 output for the fp16 QK)
            qT_sb = bigp.tile([128, 2 * TS], fp16)
            for b in range(B):
                pb, colb = (b // 2) * 64, (b % 2) * TS
                q_ps = pp.tile([128, TS], f32, tag="qk", bufs=3, name=f"q_ps{b}")
                for half in range(2):
                    tg = streamp.tile([128, SR], f32, tag="xs", bufs=3,
                                      name=f"tg{b}_{half}")
                    nc.sync.dma_start(
                        out=tg.rearrange("p (j t) -> p j t", t=TS),
                        in_=tgtT[b, half * 512:(half + 1) * 512, :]
                        .rearrange("(j p) t -> p j t", p=128))
                    for jj in range(4):
                        j = half * 4 + jj
                        nc.tensor.matmul(
                            q_ps[pb:pb + 64, :],
                            lhsT=wq_sb[:, j * DQ:(j + 1) * DQ],
                            rhs=tg[:, jj * TS:(jj + 1) * TS],
                            start=(j == 0), stop=(j == 7), tile_position=(0, pb))
                nc.scalar.activation(
                    qT_sb[pb:pb + 64, colb:colb + TS], q_ps[pb:pb + 64, :],
                    AF.Identity, bias=bq_sb[pb:pb + 64, :])

            # attention main loop: batch-pair outer so the QK psum tile can
            # triple-buffer (3 x 2 banks) against the DVE/ACT consumers.
            for pair in range(2):
                pb = pair * 64
                pv_ps = [pp.tile([65, TS], f32, tag=f"pv{h}",
                                 name=f"pv_ps{pair}_{h}") for h in range(2)]
                for sg in range(SB):
                    qkt = pp.tile([128, 2 * TS], f32, tag="qk", bufs=3,
                                  name=f"qkt{pair}_{sg}")
                    for half in range(2):
                        nc.tensor.matmul(
                            qkt[:, half * TS:(half + 1) * TS],
                            lhsT=kT2[pb:pb + 64, half * S + sg * 128:
                                     half * S + sg * 128 + 128],
                            rhs=qT_sb[pb:pb + 64, half * TS:(half + 1) * TS],
                            start=True, stop=True, tile_position=(pb, 0))
                    es = streamp.tile([128, 2 * TS], f32, tag="E", bufs=4,
                                      name=f"es{pair}_{sg}")
                    nc.vector.tensor_add(
                        es[:], qkt[:],
                        maskTd[:, sg * 2 * TS:(sg + 1) * 2 * TS])
                    pt = streamp.tile([128, 2 * TS], fp16, tag="P", bufs=6,
                                      name=f"pt{pair}_{sg}")
                    nc.scalar.activation(pt[:], es[:], AF.Exp, scale=0.125)
                    for half in range(2):
                        b = pair * 2 + half
                        kg = b * SB + sg
                        nc.tensor.matmul(
                            pv_ps[half][:],
                            lhsT=v2[:, kg * (DQ + 1):(kg + 1) * (DQ + 1)],
                            rhs=pt[:, half * TS:(half + 1) * TS],
                            start=(sg == 0), stop=(sg == SB - 1))

                # epilogue: out^T = pv[0:64]/sums + bv, all PE-free
                for half in range(2):
                    b = pair * 2 + half
                    sums = streamp.tile([65, TS], f32, tag="sums")
                    nc.scalar.copy(sums[64:65, :], pv_ps[half][64:65, :])
                    sums0 = streamp.tile([1, TS], f32, tag="sums0")
                    nc.sync.dma_start(out=sums0[:], in_=sums[64:65, :])
                    recip = streamp.tile([1, TS], f32, tag="recip")
                    rscr = streamp.tile([1, TS], f32, tag="rscr")
                    nc.vector.reciprocal_approx_accurate(recip[:], sums0[:],
                                                         rscr[:])
                    rb = streamp.tile([64, TS], f32, tag="rb")
                    nc.gpsimd.partition_broadcast(rb[:], recip[:])
                    ot = streamp.tile([64, TS], f32, tag="ot")
                    nc.vector.tensor_mul(ot[:], pv_ps[half][0:64, :], rb[:])
                    of = streamp.tile([64, TS], f32, tag="of")
                    nc.scalar.activation(of[:], ot[:], AF.Identity, bias=bv_sb[:])
                    nc.gpsimd.dma_start(out=out[b], in_=of[:])
    nc.compile()
    return nc


def _get_l1():
    if "l1" not in _CACHE:
        _CACHE["l1"] = _build_l1()
    return _CACHE["l1"]


def _get_l2():
    if "l2" not in _CACHE:
        _CACHE["l2"] = _build_l2()
    return _CACHE["l2"]


def make_in_maps_l1(src, wk, wv):
    src_flat = np.ascontiguousarray(src, dtype=F32).reshape(B * S, D)
    wk = np.ascontiguousarray(wk, dtype=F32)
    wv = np.ascontiguousarray(wv, dtype=F32)
    return [{
        "srcT": np.ascontiguousarray(src_flat[c * SR:(c + 1) * SR, :].T),
        "wk": wk, "wv": wv,
    } for c in CORES]


def glue_l1_outputs(results):
    """Assemble full kT2 / v65 arrays from the 8 per-core L1 outputs."""
    kts = [np.asarray(results[c]["kt"]).reshape(DQ, 2 * 1024) for c in CORES]
    kT_full = np.concatenate(kts, axis=1)            # [64, B*S]
    kt2 = np.concatenate([kT_full[:, :2 * S], kT_full[:, 2 * S:]],
                         axis=0).astype(FP16)
    v_full = np.concatenate(
        [np.asarray(results[c]["vout"]) for c in CORES], axis=0)  # [B*S, 64]
    v65 = np.empty((B * S, DQ + 1), dtype=FP16)
    v65[:, :DQ] = v_full.astype(FP16)
    v65[:, DQ] = np.asarray(1.0, dtype=FP16)
    # rearrange to the L2 SBUF layout: [128 partitions, (block k, col c)]
    v65 = np.ascontiguousarray(
        v65.reshape(GK, 128, DQ + 1).transpose(1, 0, 2).reshape(128, -1))
    return np.ascontiguousarray(kt2), v65


def make_in_maps_l2(kt2, v65, tgt, mask, wq, bq, bv):
    tgt = np.ascontiguousarray(tgt, dtype=F32)
    mask = np.ascontiguousarray(mask, dtype=F32)
    wq = np.ascontiguousarray(wq, dtype=F32)
    bq = np.ascontiguousarray(bq, dtype=F32)
    bv = np.ascontiguousarray(bv, dtype=F32)
    return [{
        "kt2": kt2, "v65": v65,
        "tgtT": np.ascontiguousarray(
            tgt[:, c * TS:(c + 1) * TS, :].transpose(0, 2, 1)),
        "masknT": np.ascontiguousarray(mask[c * TS:(c + 1) * TS, :].T),
        "wq": wq, "bq": bq, "bv": bv,
    } for c in CORES]


def kernel(src, tgt, mask, wq, bq, wk, bk, wv, bv):
    from concourse.bass_utils import run_bass_kernel_spmd

    res1 = run_bass_kernel_spmd(_get_l1(), make_in_maps_l1(src, wk, wv),
                                core_ids=CORES)
    kt2, v65 = glue_l1_outputs(res1.results)
    res2 = run_bass_kernel_spmd(
        _get_l2(), make_in_maps_l2(kt2, v65, tgt, mask, wq, bq, bv),
        core_ids=CORES)
    out = np.empty((B, S, DQ), dtype=F32)
    for c in CORES:
        out[:, c * TS:(c + 1) * TS, :] = \
            np.asarray(res2.results[c]["out"]).transpose(0, 2, 1)
    return out



# revision 32
# speedup vs baseline: 1.4790x; 1.4790x over previous
"""Trainium2 Bass kernel for single-head cross-attention with additive mask.

Computation (matches the reference):
    q = tgt @ wq + bq
    k = src @ wk (+ bk dropped: softmax cancels a per-row constant exactly)
    v = src @ wv (bv folded into the epilogue: out = attn@v + bv)
    s = (q k^T + mask) / sqrt(DQ)
    out = softmax(s) @ v + bv

Two SPMD launches on 8 cores (the host glue between them is pure layout
shuffling -- concat / transpose / block-diagonal placement, no math):

  L1 (projections): each core projects k,v for 1/8 of the global (B*S) src
      rows and q for its L2 shard of tgt rows.  wk|wv are stacked into one
      [128,128] stationary operand so K^T and V^T come out of a single
      matmul stream (rows 0-63 = k, 64-127 = v).  All inputs are fp16
      (host-cast); rhs tiles are N=1024 wide so the PE streams at full rate.

  L2 (attention): tgt rows sharded 8 ways; core c handles tgt rows
      [c*512,(c+1)*512) of every batch so its mask slice is read from HBM
      exactly once.  Scores are built transposed (src rows on PSUM
      partitions) so the PV matmul consumes softmax weights directly.
      Per (batch-pair, src-block):
        - QK: lhsT = [k_b0; k_b1] stacked on 128 partitions against the
          block-diagonal rhs [[q_b0,0],[0,q_b1]], so both batches of a pair
          contract in full-width 128-row passes (two 512-col matmuls --
          one matmul output must fit a single 2KB PSUM bank).
        - the DVE adds the resident fp16 mask into the scores through a
          stride-0 broadcast view (one [128,2,512] add covers both batch
          halves; the mask is stored once, not duplicated).
        - ACT applies exp(0.125*x) emitting fp16 attention weights.
        - PV accumulates fp32 in PSUM; V carries a trailing ones-column so
          row 64 of the accumulator is the softmax denominator.
      Steady state is 3-way balanced: PE ~1.1us, DVE add ~1.14us, ACT exp
      ~1.14us per (pair, src-block) iteration.
      Epilogue: the sums row (partition 64) is broadcast down to partitions
      0-63 with a tiny ones-matmul on the idle PE (reciprocal_approx_* only
      works at partition base 0, and DVE lanes cannot shift partitions),
      then reciprocal + multiply + bv bias, store.  The output leaves
      transposed [B, DQ, TS]; the host flips it.
"""
import numpy as np

B, S, D, DQ = 4, 4096, 1024, 64
NCORES = 8
TS = S // NCORES            # 512 tgt rows per core per batch (L2 shard)
SR = (B * S) // NCORES      # 2048 global src rows per core (L1 shard)
SB = S // 128               # 32 src blocks per batch
GK = B * SB                 # 128 global src blocks
DQ1 = DQ + 1                # v65 block width (ones col + v)
CORES = list(range(NCORES))
F32 = np.float32
FP16 = np.float16

_CACHE = {}


def _build_l1():
    import concourse.mybir as mybir
    import concourse.tile as tile
    from concourse import bacc

    f32 = mybir.dt.float32
    fp16 = mybir.dt.float16
    AF = mybir.ActivationFunctionType

    nc = bacc.Bacc("TRN2", target_bir_lowering=False, debug=False,
                   num_devices=NCORES)
    srcT = nc.dram_tensor("srcT", [D, SR], fp16, kind="ExternalInput")
    tgtT = nc.dram_tensor("tgtT", [D, SR], fp16, kind="ExternalInput")
    wkv = nc.dram_tensor("wkv", [D, 2 * DQ], fp16, kind="ExternalInput")
    wq = nc.dram_tensor("wq", [D, DQ], fp16, kind="ExternalInput")
    bq = nc.dram_tensor("bq", [DQ], f32, kind="ExternalInput")
    # kvt rows 0-63 = k^T, rows 64-127 = v^T (s = this core's 2048 src rows)
    kvt = nc.dram_tensor("kvt", [2 * DQ, SR], fp16, kind="ExternalOutput")
    # qt cols = (b, t) for this core's 4x512 tgt rows
    qt = nc.dram_tensor("qt", [DQ, SR], fp16, kind="ExternalOutput")

    with tile.TileContext(nc) as tc:
        with (
            tc.tile_pool(name="const", bufs=1) as constp,
            tc.tile_pool(name="big", bufs=1) as bigp,
            tc.tile_pool(name="stream", bufs=2) as streamp,
            tc.tile_pool(name="pp", bufs=1, space="PSUM") as pp,
        ):
            wkv_sb = constp.tile([128, 8 * 2 * DQ], fp16)
            nc.sync.dma_start(
                out=wkv_sb.rearrange("p (j m) -> p j m", m=2 * DQ),
                in_=wkv.rearrange("(j p) m -> p j m", p=128))
            wq_sb = constp.tile([128, 8 * DQ], fp16)
            nc.sync.dma_start(
                out=wq_sb.rearrange("p (j m) -> p j m", m=DQ),
                in_=wq.rearrange("(j p) m -> p j m", p=128))
            bq_sb = constp.tile([DQ, 1], f32)
            nc.sync.dma_start(out=bq_sb[:], in_=bq.rearrange("(p o) -> p o", o=1))

            kv_ps = pp.tile([128, SR], f32, tag="kv")
            q_ps = pp.tile([DQ, SR], f32, tag="q")
            for j in range(8):
                st = streamp.tile([128, SR], fp16, tag="xs", bufs=8,
                                  name=f"st{j}")
                tg = streamp.tile([128, SR], fp16, tag="xt", bufs=8,
                                  name=f"tg{j}")
                if j < 2:
                    # fine-grained first chunks spread over many DMA queues
                    # (per-queue bandwidth is only ~20GB/s) so the PE
                    # unblocks quickly instead of waiting on whole tiles
                    engs = [nc.sync, nc.scalar, nc.gpsimd]
                    for h in range(8):
                        engs[h % 3].dma_start(
                            out=st[:, h * 256:(h + 1) * 256],
                            in_=srcT[j * 128:(j + 1) * 128,
                                     h * 256:(h + 1) * 256])
                        engs[(h + 1) % 3].dma_start(
                            out=tg[:, h * 256:(h + 1) * 256],
                            in_=tgtT[j * 128:(j + 1) * 128,
                                     h * 256:(h + 1) * 256])
                else:
                    for h in range(2):
                        nc.sync.dma_start(
                            out=st[:, h * 1024:(h + 1) * 1024],
                            in_=srcT[j * 128:(j + 1) * 128,
                                     h * 1024:(h + 1) * 1024])
                        nc.gpsimd.dma_start(
                            out=tg[:, h * 1024:(h + 1) * 1024],
                            in_=tgtT[j * 128:(j + 1) * 128,
                                     h * 1024:(h + 1) * 1024])
                for h in range(4):
                    nc.tensor.matmul(
                        kv_ps[:, h * 512:(h + 1) * 512],
                        lhsT=wkv_sb[:, j * 128:(j + 1) * 128],
                        rhs=st[:, h * 512:(h + 1) * 512],
                        start=(j == 0), stop=(j == 7))
                    nc.tensor.matmul(
                        q_ps[:, h * 512:(h + 1) * 512],
                        lhsT=wq_sb[:, j * DQ:(j + 1) * DQ],
                        rhs=tg[:, h * 512:(h + 1) * 512],
                        start=(j == 0), stop=(j == 7))
            kv_sb = bigp.tile([128, SR], fp16)
            q_sb = bigp.tile([DQ, SR], fp16)
            for h in range(2):
                sl = slice(h * 1024, (h + 1) * 1024)
                nc.vector.tensor_copy(kv_sb[:, sl], kv_ps[:, sl])
                nc.scalar.activation(q_sb[:, sl], q_ps[:, sl], AF.Identity,
                                     bias=bq_sb[:])
                nc.sync.dma_start(out=kvt[:, sl], in_=kv_sb[:, sl])
                nc.gpsimd.dma_start(out=qt[:, sl], in_=q_sb[:, sl])
    nc.compile()
    return nc


def _build_l2():
    import concourse.mybir as mybir
    import concourse.tile as tile
    from concourse import bacc

    f32 = mybir.dt.float32
    fp16 = mybir.dt.float16
    AF = mybir.ActivationFunctionType

    nc = bacc.Bacc("TRN2", target_bir_lowering=False, debug=False,
                   num_devices=NCORES)
    # kt2 cols pair*S + s; rows 0-63 = d of batch 2*pair, 64-127 = 2*pair+1
    kt2d = nc.dram_tensor("kt2", [128, 2 * S], fp16, kind="ExternalInput")
    # v65 block kg: cols 0..63 = v[kg*128 + p, :], col 64 = ones
    v65d = nc.dram_tensor("v65", [128, GK * DQ1], fp16, kind="ExternalInput")
    # block-diagonal q: qbd[pair] = [[q_b0^T, 0], [0, q_b1^T]]  (128 x 1024)
    qbdd = nc.dram_tensor("qbd", [2, 128, 2 * TS], fp16, kind="ExternalInput")
    # mask slice pre-arranged by the host into the exact SBUF layout
    # [128 partitions, (sb, t)] so the load is fully linear (4KB+ lines)
    maskPd = nc.dram_tensor("maskP", [128, SB * TS], fp16,
                            kind="ExternalInput")
    bv = nc.dram_tensor("bv", [DQ], f32, kind="ExternalInput")
    # transposed fp16 output: host flips [B, DQ, TS] -> [B, TS, DQ] and
    # upcasts to fp32 (pure layout/dtype glue)
    out = nc.dram_tensor("out", [B, DQ, TS], fp16, kind="ExternalOutput")

    with tile.TileContext(nc) as tc:
        with (
            tc.tile_pool(name="const", bufs=1) as constp,
            tc.tile_pool(name="big", bufs=1) as bigp,
            tc.tile_pool(name="stream", bufs=2) as streamp,
            tc.tile_pool(name="pp", bufs=1, space="PSUM") as pp,
        ):
            bv_sb = constp.tile([DQ, 1], f32)
            nc.sync.dma_start(out=bv_sb[:], in_=bv.rearrange("(p o) -> p o", o=1))
            # ones row at partition 64 (same base as the pv sums row)
            ones_sb = constp.tile([DQ1, DQ], f32)
            nc.vector.memset(ones_sb[DQ:DQ1, :], 1.0)

            # Resident loads, ordered so iteration 0 unblocks ASAP and each
            # src-block's data lands ahead of its loop iteration.
            qbd_sb = bigp.tile([128, 2 * 2 * TS], fp16)
            kt2 = bigp.tile([128, 2 * S], fp16)
            v65 = bigp.tile([128, GK * DQ1], fp16)
            # mask resident once; the DVE add reads it through a stride-0
            # broadcast view to cover both batch halves of a score tile
            msb = bigp.tile([128, SB * TS], fp16)

            def load_kt2(eng, c0, c1):
                eng.dma_start(out=kt2[:, c0:c1], in_=kt2d[:, c0:c1])

            def load_v65(eng, k0, k1):
                eng.dma_start(out=v65[:, k0 * DQ1:k1 * DQ1],
                              in_=v65d[:, k0 * DQ1:k1 * DQ1])

            def load_mask(eng, s0, s1):
                eng.dma_start(out=msb[:, s0 * TS:s1 * TS],
                              in_=maskPd[:, s0 * TS:s1 * TS])

            # iteration-0 critical chunks first, small and spread across
            # many DMA queues (per-queue bandwidth is only ~20GB/s)
            for i in range(4):
                eng = [nc.sync, nc.scalar, nc.gpsimd, nc.sync][i]
                eng.dma_start(
                    out=qbd_sb[:, i * 512:(i + 1) * 512],
                    in_=qbdd[i // 2, :, (i % 2) * 512:(i % 2 + 1) * 512])
            load_kt2(nc.scalar, 0, 256)      # pair0: sg 0-1
            load_mask(nc.gpsimd, 0, 1)
            load_v65(nc.sync, 0, 2)          # b0: kg 0-1
            load_v65(nc.scalar, 32, 34)      # b1: kg 32-33
            # near-term chunks
            load_mask(nc.gpsimd, 1, 4)
            load_kt2(nc.scalar, 256, 1024)
            load_v65(nc.sync, 2, 8)
            load_v65(nc.scalar, 34, 40)
            load_mask(nc.gpsimd, 4, 8)
            # bulk, in consumption order
            load_kt2(nc.sync, 1024, 4096)    # pair0 rest
            load_v65(nc.gpsimd, 8, 32)
            load_v65(nc.gpsimd, 40, 64)
            load_mask(nc.gpsimd, 8, 20)
            load_mask(nc.gpsimd, 20, 32)
            load_kt2(nc.sync, 4096, 8192)    # pair1
            load_v65(nc.sync, 64, 128)

            # main loop: batch-pair outer; scores stay transposed
            for pair in range(2):
                pv = [pp.tile([DQ1, TS], f32, tag=f"pv{h}",
                              name=f"pv{pair}_{h}") for h in range(2)]
                for sg in range(SB):
                    qkt = pp.tile([128, 2 * TS], f32, tag="qk", bufs=3,
                                  name=f"qkt{pair}_{sg}")
                    for half in range(2):
                        nc.tensor.matmul(
                            qkt[:, half * TS:(half + 1) * TS],
                            lhsT=kt2[:, pair * S + sg * 128:
                                     pair * S + sg * 128 + 128],
                            rhs=qbd_sb[:, pair * 1024 + half * TS:
                                       pair * 1024 + (half + 1) * TS],
                            start=True, stop=True)
                    es = streamp.tile([128, 2 * TS], fp16, tag="E", bufs=4,
                                      name=f"es{pair}_{sg}")
                    nc.vector.tensor_add(
                        es.rearrange("p (h t) -> p h t", h=2),
                        qkt.rearrange("p (h t) -> p h t", h=2),
                        msb[:, None, sg * TS:(sg + 1) * TS]
                        .broadcast_to([128, 2, TS]))
                    pt = streamp.tile([128, 2 * TS], fp16, tag="P", bufs=4,
                                      name=f"pt{pair}_{sg}")
                    nc.scalar.activation(pt[:], es[:], AF.Exp, scale=0.125)
                    for half in range(2):
                        kg = (pair * 2 + half) * SB + sg
                        nc.tensor.matmul(
                            pv[half][:],
                            lhsT=v65[:, kg * DQ1:(kg + 1) * DQ1],
                            rhs=pt[:, half * TS:(half + 1) * TS],
                            start=(sg == 0), stop=(sg == SB - 1))

                # epilogue: out^T = pv[0:64]/pv[64] + bv.  The sums row sits
                # on partition 64 but reciprocal_approx_* only works at
                # partition base 0 (custom-DVE uop), and DVE lanes can't
                # shift partitions -- so broadcast the sums down to
                # partitions 0-63 with a tiny ones-matmul on the idle PE,
                # then reciprocal + multiply lane-aligned with the values.
                for half in range(2):
                    b = pair * 2 + half
                    pvs = streamp.tile([DQ1, TS], f32, tag="pvs",
                                       name=f"pvs{b}")
                    nc.scalar.copy(pvs[:], pv[half][:])
                    rb = pp.tile([DQ, TS], f32, tag="pv0", bufs=1,
                                 name=f"rb{b}")
                    nc.tensor.matmul(rb[:], lhsT=ones_sb[DQ:DQ1, :],
                                     rhs=pvs[DQ:DQ1, :],
                                     start=True, stop=True)
                    sums_sb = streamp.tile([DQ, TS], f32, tag="sums",
                                           name=f"sums{b}")
                    nc.scalar.copy(sums_sb[:], rb[:])
                    recip = streamp.tile([DQ, TS], f32, tag="recip",
                                         name=f"recip{b}")
                    nc.vector.reciprocal_approx_fast(recip[:], sums_sb[:])
                    ot = streamp.tile([DQ, TS], f32, tag="ot", name=f"ot{b}")
                    nc.vector.tensor_mul(ot[:], pvs[0:DQ, :], recip[:])
                    of = streamp.tile([DQ, TS], fp16, tag="of",
                                      name=f"of{b}")
                    nc.scalar.activation(of[:], ot[:], AF.Identity,
                                         bias=bv_sb[:])
                    nc.gpsimd.dma_start(out=out[b], in_=of[:])
    nc.compile()
    return nc


def _get_l1():
    if "l1" not in _CACHE:
        _CACHE["l1"] = _build_l1()
    return _CACHE["l1"]


def _get_l2():
    if "l2" not in _CACHE:
        _CACHE["l2"] = _build_l2()
    return _CACHE["l2"]


def make_in_maps_l1(src, tgt, wk, wv, wq, bq):
    src_flat = np.asarray(src, dtype=F32).reshape(B * S, D)
    wkv = np.concatenate([np.asarray(wk, dtype=F32),
                          np.asarray(wv, dtype=F32)], axis=1).astype(FP16)
    wq16 = np.asarray(wq, dtype=F32).astype(FP16)
    bq = np.ascontiguousarray(bq, dtype=F32)
    tgt = np.asarray(tgt, dtype=F32)
    maps = []
    for c in CORES:
        # tgtT cols (b, t) for this core's L2 shard of tgt rows
        tslice = tgt[:, c * TS:(c + 1) * TS, :]         # [B, TS, D]
        tgtT = tslice.transpose(2, 0, 1).reshape(D, B * TS)
        maps.append({
            "srcT": np.ascontiguousarray(
                src_flat[c * SR:(c + 1) * SR, :].T.astype(FP16)),
            "tgtT": np.ascontiguousarray(tgtT.astype(FP16)),
            "wkv": wkv, "wq": wq16, "bq": bq,
        })
    return maps


def glue_l1_outputs(results):
    """Assemble L2's kt2 / v65 / per-core qbd from the 8 L1 outputs."""
    kvs = [np.asarray(results[c]["kvt"]) for c in CORES]
    kT_full = np.concatenate([kv[0:DQ] for kv in kvs], axis=1)    # [64, B*S]
    vT_full = np.concatenate([kv[DQ:2 * DQ] for kv in kvs], axis=1)
    # kt2: [128, 2S]; cols pair*S+s; rows 0-63 = batch 2p, 64-127 = 2p+1
    kt2 = np.empty((128, 2 * S), dtype=FP16)
    for pair in range(2):
        kt2[0:DQ, pair * S:(pair + 1) * S] = \
            kT_full[:, (2 * pair) * S:(2 * pair + 1) * S]
        kt2[DQ:128, pair * S:(pair + 1) * S] = \
            kT_full[:, (2 * pair + 1) * S:(2 * pair + 2) * S]
    v_full = vT_full.T                                            # [B*S, 64]
    v65 = np.empty((B * S, DQ1), dtype=FP16)
    v65[:, :DQ] = v_full
    v65[:, DQ] = np.asarray(1.0, dtype=FP16)
    v65 = np.ascontiguousarray(
        v65.reshape(GK, 128, DQ1).transpose(1, 0, 2).reshape(128, -1))
    # per-core block-diagonal q
    qbds = []
    for c in CORES:
        q = np.asarray(results[c]["qt"])                          # [64, B*TS]
        qbd = np.zeros((2, 128, 2 * TS), dtype=FP16)
        for bt in range(B):
            pair, h = bt // 2, bt % 2
            qbd[pair, h * DQ:(h + 1) * DQ, h * TS:(h + 1) * TS] = \
                q[:, bt * TS:(bt + 1) * TS]
        qbds.append(qbd)
    return np.ascontiguousarray(kt2), v65, qbds


def make_in_maps_l2(kt2, v65, qbds, mask, bv):
    mask = np.asarray(mask, dtype=F32)
    bv = np.ascontiguousarray(bv, dtype=F32)
    maps = []
    for c in CORES:
        mT = mask[c * TS:(c + 1) * TS, :].T.astype(FP16)   # [S, TS]
        mP = np.ascontiguousarray(
            mT.reshape(SB, 128, TS).transpose(1, 0, 2).reshape(128, SB * TS))
        maps.append({"kt2": kt2, "v65": v65, "qbd": qbds[c],
                     "maskP": mP, "bv": bv})
    return maps


def kernel(src, tgt, mask, wq, bq, wk, bk, wv, bv):
    from concourse.bass_utils import run_bass_kernel_spmd

    res1 = run_bass_kernel_spmd(
        _get_l1(), make_in_maps_l1(src, tgt, wk, wv, wq, bq), core_ids=CORES)
    kt2, v65, qbds = glue_l1_outputs(res1.results)
    res2 = run_bass_kernel_spmd(
        _get_l2(), make_in_maps_l2(kt2, v65, qbds, mask, bv), core_ids=CORES)
    out = np.empty((B, S, DQ), dtype=F32)
    for c in CORES:
        out[:, c * TS:(c + 1) * TS, :] = \
            np.asarray(res2.results[c]["out"]).transpose(0, 2, 1)
    return out
